# revision 45
# baseline (speedup 1.0000x reference)
"""Trainium2 Bass kernel for nn_CMF: per-channel spatial row-attention + 1x1 convs.

Reference (B=16, C=768, H=W=56):
  q = Wq @ x_s ; k = Wk @ x_fq ; v = Wv @ x_fq        (1x1 convs)
  scores[b,c,h,g] = sum_w q[b,c,h,w] k[b,c,g,w] * (H*W*C)**-0.5
  attn = softmax(scores, -1); fuse = attn @ v
  out = W1 @ zero_pad(x_s + x_mt + fuse, 1) + b1      -> (B, C, 58, 58)

Sharding: data-parallel over batch; 2 images per core on 8 cores (SPMD).

Per-core design (v4):
  - x_s / x_fq resident in SBUF as fp8e4 (one DRAM read per image);
    wq/wk/wv fp8 with DoubleRow mix matmuls (256-deep contraction per
    instruction), w1 bf16.
  - Per 128-channel block o: q/v mix matmuls -> staging in "pair layout"
    (channels 0-63 write w at free slots 0..55, channels 64-127 at
    64..119); k mix reuses the q staging tile after its transpose.
    Staging tiles are PERSISTENT and zero-initialized once: slots
    56:64 / 120:128 stay zero forever, so full-width [128, *] softmax
    ops are safe (garbage rows are never amplified into used lanes).
  - One xbar DMA-transpose per matrix -> QT/KT [wslot, h, c] and
    VH [hslot, w, c]; channel pair (c, 64+c) then runs attention as
    concurrent PE quadrant matmuls: A at partitions 0-55 / psum 0-55,
    B at partitions 64-119 / psum 64-119 (tile_position (0,0)/(64,64)).
  - softmax per group: ONE exp on ACT over all 128 partitions;
    denominators via ONE block-diagonal all-ones matmul (den_lo ->
    psum 0:64, den_hi -> 64:128); 1/sum via ln+exp on ACT; ONE
    pre-normalize multiply on DVE.
  - fuse -> FT2 [wslot, h, c] -> back-xbar -> fn2 [c, h, wslot] ->
    DRAM bounce in padded layout (contiguous 14KB/partition descriptors).
  - Phase C: s = x_s + x_mt via bf16 gpsimd accum DMA; fuse added with
    dual-AP adds split across DVE (lo) and GpSimd (hi); conv + bias on
    ACT; y rows written as flattened [128, 464] single-descriptor DMAs.
"""

import sys

import numpy as np

sys.path.insert(0, "/opt/trn_rl_repo")

N_CORES = 8


class Cfg:
    def __init__(self, imgs=2, cb=6, h=56, w=56, rt=8):
        self.imgs = imgs
        self.cb = cb
        self.C = cb * 128
        self.H = h
        self.W = w
        self.S = h * w
        self.RT = rt
        assert h % rt == 0
        self.NT = h // rt
        self.NS = rt * w
        assert self.NS * 4 <= 2048  # one PSUM bank
        self.GC = 8  # channel pairs per attention group
        assert 64 % self.GC == 0
        self.NGRP = 64 // self.GC
        self.scale = float((h * w * self.C) ** -0.5)
        self.HP = h + 2
        self.WP = w + 2
        assert h <= 56 and w <= 56  # pair layout needs w,h <= 64; 64+h <= 128


def build_program(cfg):
    from contextlib import ExitStack

    import concourse.bass as bass
    import concourse.mybir as mybir
    import concourse.tile as tile

    f32 = mybir.dt.float32
    bf16 = mybir.dt.bfloat16
    fp8 = mybir.dt.float8e4
    AF = mybir.ActivationFunctionType
    ALU = mybir.AluOpType
    DR = mybir.MatmulPerfMode.DoubleRow

    nc = bass.Bass()

    IM, CB, H, W, S = cfg.imgs, cfg.cb, cfg.H, cfg.W, cfg.S
    RT, NT, NS, C = cfg.RT, cfg.NT, cfg.NS, cfg.C
    HP, WP = cfg.HP, cfg.WP
    GC, NGRP = cfg.GC, cfg.NGRP

    x_s = nc.declare_dram_parameter("x_s", [IM, C, S], f32, isOutput=False)
    x_fq = nc.declare_dram_parameter("x_fq", [IM, C, S], f32, isOutput=False)
    x_mt = nc.declare_dram_parameter("x_mt", [IM, C, S], f32, isOutput=False)
    wqT = nc.declare_dram_parameter("wqT", [C, C], f32, isOutput=False)
    wkT = nc.declare_dram_parameter("wkT", [C, C], f32, isOutput=False)
    wvT = nc.declare_dram_parameter("wvT", [C, C], f32, isOutput=False)
    w1T = nc.declare_dram_parameter("w1T", [C, C], f32, isOutput=False)
    b1 = nc.declare_dram_parameter("b1", [C], f32, isOutput=False)
    y = nc.declare_dram_parameter("y", [IM, C, HP, WP], f32, isOutput=True)

    # fuse bounce buffer, padded layout: [img, ob, c, h, wslot(128)]
    fnat_d = nc.dram_tensor("fnat_d", [IM, CB, 128, H, 128], bf16)

    with tile.TileContext(nc) as tc, ExitStack() as ex:
        wpool = ex.enter_context(tc.tile_pool(name="wpool", bufs=1))
        xrpool = ex.enter_context(tc.tile_pool(name="xrpool", bufs=1))
        tp = ex.enter_context(tc.tile_pool(name="tp", bufs=2))
        ftp = ex.enter_context(tc.tile_pool(name="ftp", bufs=1))
        ep = ex.enter_context(tc.tile_pool(name="ep", bufs=2))
        yrp = ex.enter_context(tc.tile_pool(name="yrp", bufs=3))
        sbp = ex.enter_context(tc.tile_pool(name="sbp", bufs=2))
        mps = ex.enter_context(tc.tile_pool(name="mps", bufs=1, space="PSUM"))
        aps = ex.enter_context(tc.tile_pool(name="aps", bufs=3, space="PSUM"))

        # ---- resident weights / constants ----
        wq = wpool.tile([128, CB, C], fp8)
        wk = wpool.tile([128, CB, C], fp8)
        wv = wpool.tile([128, CB, C], fp8)
        w1 = wpool.tile([128, CB, C], bf16)
        for dst, src in ((wq, wqT), (wk, wkT), (wv, wvT), (w1, w1T)):
            nc.gpsimd.dma_start(
                out=dst, in_=src.rearrange("(kb p) o -> p kb o", p=128))
        b1t = wpool.tile([128, CB], f32)
        nc.gpsimd.dma_start(out=b1t, in_=b1.rearrange("(kb p) -> p kb", p=128))
        bord = wpool.tile([128, CB, WP], bf16)
        nc.vector.tensor_copy(
            out=bord,
            in_=bass.AP(tensor=b1t.tensor, offset=b1t.offset,
                        ap=[list(b1t.ap[0]), list(b1t.ap[1]), [0, WP]]))
        # block-diagonal all-ones stationary: den_lo -> psum 0:64,
        # den_hi -> psum 64:128, in ONE matmul
        onesd = wpool.tile([128, 128], bf16, name="onesd")
        nc.vector.memset(onesd, 0.0)
        nc.vector.memset(onesd[0:H, 0:64], 1.0)
        nc.vector.memset(onesd[64:64 + H, 64:128], 1.0)
        # persistent pair-layout staging (k/q share one tile; each mix's
        # transpose fires as soon as that mix's copies land, so the ring
        # starts early).  Zero-init once: slots 56:64 / 120:128 are
        # never written again.
        qsg = wpool.tile([128, H, 128], bf16, name="qsg")
        vstg = wpool.tile([128, W, 128], bf16, name="vsg")
        nc.vector.memset(qsg, 0.0)
        nc.vector.memset(vstg, 0.0)
        # zero the never-written partition rows of the rotating scores
        # psum banks ONCE: matmul start= only resets addressed rows, so
        # these stay 0 and exp() of them feeds the onesd contraction a
        # harmless 1.0 instead of potential uninitialized-psum NaN.
        for _i in range(3):
            tz = aps.tile([128, GC * H], f32, tag="att")
            nc.vector.memset(tz[32:64], 0.0)
            nc.vector.memset(tz[96:128], 0.0)

        for img in range(IM):
            # ---- resident x (fp8) ----
            with nc.named_scope("xload"):
                xrs = xrpool.tile([128, CB, S], fp8, tag="xs",
                                  name=f"xrs_{img}")
                xrf = xrpool.tile([128, CB, S], fp8, tag="xf",
                                  name=f"xrf_{img}")
                xs_r = x_s[img].rearrange("(kb p) s -> p kb s", p=128)
                xf_r = x_fq[img].rearrange("(kb p) s -> p kb s", p=128)
                # chunked so the o=0 mix matmuls can start immediately
                for n in range(NT):
                    nsl = slice(n * NS, (n + 1) * NS)
                    nc.gpsimd.dma_start(out=xrs[:, :, nsl], in_=xs_r[:, :, nsl])
                    nc.gpsimd.dma_start(out=xrf[:, :, nsl], in_=xf_r[:, :, nsl])

            for o in range(CB):
                osl = slice(o * 128, (o + 1) * 128)
                # ---- phase A: channel mix into pair-layout staging ----
                # order v, k, q: each matrix's xbar transpose is issued
                # right after its staging copies, so the (serial) ring
                # overlaps the next matrix's mix matmuls.  k and q share
                # the staging tile; q's copies wait only on KT's read.
                # NOTE: all xbar transposes must stay on ONE HWDGE ring —
                # concurrent transposes from both rings corrupt data
                # (shared xbar unit; verified on HW).
                QT = tp.tile([128, H, 128], bf16, tag="qt",
                             name=f"qt_{img}_{o}")
                VH = tp.tile([128, W, 128], bf16, tag="vh", bufs=1,
                             name=f"vh_{img}_{o}")
                KT = tp.tile([128, H, 128], bf16, tag="kt",
                             name=f"kt_{img}_{o}")
                with nc.named_scope("mix"):
                    for kind, wm, xr, tag in (
                            ("k", wk, xrf, "vp"), ("q", wq, xrs, "qp"),
                            ("v", wv, xrf, "vp")):
                        for n in range(NT):
                            nsl = slice(n * NS, (n + 1) * NS)
                            rsl = slice(n * RT, (n + 1) * RT)
                            ps = mps.tile([128, NS], f32, tag=tag, bufs=2)
                            for kb in range(0, CB, 2):
                                nc.tensor.matmul(
                                    ps, lhsT=wm[:, kb:kb + 2, osl],
                                    rhs=xr[:, kb:kb + 2, nsl],
                                    perf_mode=DR,
                                    start=(kb == 0), stop=(kb == CB - 2))
                            if kind == "v":
                                nc.vector.tensor_copy(
                                    out=vstg[0:64, 0:W, rsl],
                                    in_=ps[0:64].rearrange(
                                        "p (r w) -> p w r", r=RT))
                                nc.vector.tensor_copy(
                                    out=vstg[64:128, 0:W,
                                             64 + n * RT:64 + (n + 1) * RT],
                                    in_=ps[64:128].rearrange(
                                        "p (r w) -> p w r", r=RT))
                            else:
                                nc.vector.tensor_copy(
                                    out=qsg[0:64, rsl, 0:W],
                                    in_=ps[0:64].rearrange(
                                        "p (r w) -> p r w", r=RT))
                                nc.vector.tensor_copy(
                                    out=qsg[64:128, rsl, 64:64 + W],
                                    in_=ps[64:128].rearrange(
                                        "p (r w) -> p r w", r=RT))
                        with nc.named_scope("xpose"):
                            if kind == "v":
                                nc.sync.dma_start(
                                    out=VH,
                                    in_=vstg.rearrange("p a b -> p (a b)"),
                                    transpose=True)
                            elif kind == "k":
                                nc.sync.dma_start(
                                    out=KT,
                                    in_=qsg.rearrange("p a b -> p (a b)"),
                                    transpose=True)
                            else:
                                nc.sync.dma_start(
                                    out=QT,
                                    in_=qsg.rearrange("p a b -> p (a b)"),
                                    transpose=True)

                # ---- phase B: paired per-channel attention ----
                FT2 = ftp.tile([128, H, 128], bf16, tag="ft",
                               name=f"ft_{img}_{o}")
                with nc.named_scope("attn"):
                    for g in range(NGRP):
                        sp = aps.tile([128, GC * H], f32, tag="att")
                        for ci in range(GC):
                            j = g * GC + ci
                            csl = slice(ci * H, (ci + 1) * H)
                            nc.tensor.matmul(
                                sp[0:W, csl], lhsT=KT[0:W, :, j],
                                rhs=QT[0:W, :, j], start=True, stop=True)
                            nc.tensor.matmul(
                                sp[64:64 + W, csl], lhsT=KT[64:64 + W, :, 64 + j],
                                rhs=QT[64:64 + W, :, 64 + j],
                                start=True, stop=True)
                        # one exp over all 128 partitions: rows 56:64 /
                        # 120:128 hold stale psum (finite); exp of them is
                        # never read by bp (onesd rows are 0 there) nor by
                        # the fuse matmuls (contraction 0:56 / 64:120).
                        et = ep.tile([128, GC * H], bf16, tag="et", bufs=1)
                        nc.scalar.activation(
                            out=et, in_=sp, func=AF.Exp, scale=cfg.scale)
                        bp = aps.tile([128, GC * H], f32, tag="bp", bufs=1)
                        nc.tensor.matmul(bp, lhsT=onesd, rhs=et,
                                         start=True, stop=True)
                        # 1/sum via exp(-ln(sum)) on ACT; second step in
                        # place (tolerance here is ~2e-2)
                        rt = ep.tile([128, GC * H], f32, tag="rt", bufs=1)
                        nc.scalar.activation(out=rt, in_=bp, func=AF.Ln)
                        nc.scalar.activation(out=rt, in_=rt, func=AF.Exp,
                                             scale=-1.0)
                        en = ep.tile([128, GC * H], bf16, tag="en", bufs=1)
                        nc.vector.tensor_tensor(
                            out=en, in0=et, in1=rt, op=ALU.mult)
                        fp = aps.tile([128, GC * H], f32, tag="att")
                        for ci in range(GC):
                            j = g * GC + ci
                            csl = slice(ci * H, (ci + 1) * H)
                            nc.tensor.matmul(
                                fp[0:W, csl], lhsT=VH[0:H, :, j],
                                rhs=en[0:H, csl], start=True, stop=True)
                            nc.tensor.matmul(
                                fp[64:64 + W, csl], lhsT=VH[64:64 + H, :, 64 + j],
                                rhs=en[64:64 + H, csl], start=True, stop=True)
                        # contiguous-out cast (c is FT2's innermost dim)
                        nc.vector.tensor_copy(
                            out=FT2[0:W, :, g * GC:(g + 1) * GC],
                            in_=fp[0:W].rearrange("p (c h) -> p h c", c=GC))
                        nc.scalar.copy(
                            out=FT2[64:64 + W, :,
                                    64 + g * GC:64 + (g + 1) * GC],
                            in_=fp[64:64 + W].rearrange(
                                "p (c h) -> p h c", c=GC))
                with nc.named_scope("xback"):
                    fn2 = ftp.tile([128, H, 128], bf16, tag="fn",
                                   name=f"fn_{img}_{o}")
                    nc.sync.dma_start(
                        out=fn2, in_=FT2.rearrange("p a b -> p (a b)"),
                        transpose=True)
                    nc.scalar.dma_start(out=fnat_d[img, o], in_=fn2)

            # ---- phase C: s-add + conv + y assembly ----
            with nc.named_scope("conv"):
                xs_i = x_s[img].rearrange("(kb p) s -> p kb s", p=128)
                xmt_i = x_mt[img].rearrange("(kb p) s -> p kb s", p=128)
                fn_i = fnat_d[img].rearrange("kb p h w -> p kb h w")
                for n in range(NT):
                    nsl = slice(n * NS, (n + 1) * NS)
                    rsl = slice(n * RT, (n + 1) * RT)
                    s0 = tp.tile([128, CB, NS], bf16, tag="qt",
                                 name=f"s0_{img}_{n}")
                    nc.gpsimd.dma_start(out=s0, in_=xs_i[:, :, nsl])
                    nc.gpsimd.dma_start(out=s0, in_=xmt_i[:, :, nsl],
                                        accum_op=ALU.add)
                    fr2 = tp.tile([128, CB, RT, 128], bf16, tag="kt",
                                  name=f"fr_{img}_{n}")
                    nc.scalar.dma_start(out=fr2, in_=fn_i[:, :, rsl, :])
                    s0b = sbp.tile([128, CB, NS], bf16, tag="s0b", bufs=2)
                    nc.vector.tensor_tensor(
                        out=s0b[0:64].rearrange("p kb (r w) -> p kb r w", r=RT),
                        in0=s0[0:64].rearrange("p kb (r w) -> p kb r w", r=RT),
                        in1=fr2[0:64, :, :, 0:W], op=ALU.add)
                    nc.vector.tensor_tensor(
                        out=s0b[64:128].rearrange(
                            "p kb (r w) -> p kb r w", r=RT),
                        in0=s0[64:128].rearrange(
                            "p kb (r w) -> p kb r w", r=RT),
                        in1=fr2[64:128, :, :, 64:64 + W], op=ALU.add)
                    for o in range(CB):
                        pc = mps.tile([128, NS], f32, tag="qp", bufs=2)
                        for kb in range(CB):
                            nc.tensor.matmul(
                                pc, lhsT=w1[:, kb, o * 128:(o + 1) * 128],
                                rhs=s0b[:, kb, :],
                                start=(kb == 0), stop=(kb == CB - 1))
                        yr = yrp.tile([128, RT, WP], f32, tag="yr", bufs=2)
                        # border columns 0 and WP-1 <- b1
                        bcol = bord[:, o:o + 1, 0:RT].rearrange(
                            "p a b -> p b a")
                        nc.vector.tensor_copy(out=yr[:, :, 0:1], in_=bcol)
                        nc.vector.tensor_copy(
                            out=yr[:, :, WP - 1:WP], in_=bcol)
                        nc.scalar.activation(
                            out=yr[:, :, 1:1 + W],
                            in_=pc.rearrange("p (r w) -> p r w", r=RT),
                            func=AF.Identity, bias=b1t[:, o:o + 1])
                        yeng = (nc.scalar, nc.sync, nc.gpsimd)[o % 3]
                        yeng.dma_start(
                            out=y[img, o * 128:(o + 1) * 128,
                                  1 + n * RT:1 + (n + 1) * RT, :].rearrange(
                                      "c r w -> c (r w)"),
                            in_=yr.rearrange("p r w -> p (r w)"))
                for o in range(CB):
                    # bf16 bord -> f32 y needs the casting (gpsimd) path
                    yo = y[img, o * 128:(o + 1) * 128]
                    nc.gpsimd.dma_start(out=yo[:, 0, :], in_=bord[:, o, :])
                    nc.gpsimd.dma_start(out=yo[:, HP - 1, :], in_=bord[:, o, :])

    import bass_rust as _bass_rust
    _bass_rust.move_matmul_waits_to_ldweights(nc.m)
    _bass_rust.generate_event_semaphores(nc)
    return nc


_PROG_CACHE = {}


def get_program():
    if "full" not in _PROG_CACHE:
        _PROG_CACHE["full"] = build_program(Cfg())
    return _PROG_CACHE["full"]


def _prep_in_maps(x_s, x_fq, x_mt, Wq, Wk, Wv, W1, b1):
    x_s = np.asarray(x_s, dtype=np.float32)
    x_fq = np.asarray(x_fq, dtype=np.float32)
    x_mt = np.asarray(x_mt, dtype=np.float32)
    wqT = np.ascontiguousarray(np.asarray(Wq, np.float32).T)
    wkT = np.ascontiguousarray(np.asarray(Wk, np.float32).T)
    wvT = np.ascontiguousarray(np.asarray(Wv, np.float32).T)
    w1T = np.ascontiguousarray(np.asarray(W1, np.float32).T)
    b1 = np.asarray(b1, dtype=np.float32)

    B, C, H, W = x_s.shape
    per = B // N_CORES
    in_maps = []
    for i in range(N_CORES):
        sl = slice(i * per, (i + 1) * per)
        in_maps.append({
            "x_s": np.ascontiguousarray(x_s[sl].reshape(per, C, H * W)),
            "x_fq": np.ascontiguousarray(x_fq[sl].reshape(per, C, H * W)),
            "x_mt": np.ascontiguousarray(x_mt[sl].reshape(per, C, H * W)),
            "wqT": wqT, "wkT": wkT, "wvT": wvT, "w1T": w1T, "b1": b1,
        })
    return in_maps, per, C, H, W


def kernel(x_s, x_fq, x_mt, Wq, Wk, Wv, W1, b1, trace=False):
    from concourse.bass_utils import run_bass_kernel_spmd

    in_maps, per, C, H, W = _prep_in_maps(
        x_s, x_fq, x_mt, Wq, Wk, Wv, W1, b1)
    nc = get_program()
    r = run_bass_kernel_spmd(nc, in_maps, list(range(N_CORES)), trace=trace)
    out = np.concatenate(
        [r.results[i]["y"].reshape(per, C, H + 2, W + 2)
         for i in range(N_CORES)], axis=0).astype(np.float32)
    if trace:
        return out, r
    return out


# revision 49
# speedup vs baseline: 1.0825x; 1.0825x over previous
"""Trainium2 Bass kernel for nn_CMF: per-channel spatial row-attention + 1x1 convs.

Reference (B=16, C=768, H=W=56):
  q = Wq @ x_s ; k = Wk @ x_fq ; v = Wv @ x_fq        (1x1 convs)
  scores[b,c,h,g] = sum_w q[b,c,h,w] k[b,c,g,w] * (H*W*C)**-0.5
  attn = softmax(scores, -1); fuse = attn @ v
  out = W1 @ zero_pad(x_s + x_mt + fuse, 1) + b1      -> (B, C, 58, 58)

Sharding: data-parallel over batch; 2 images per core on 8 cores (SPMD).

Per-core design (v4):
  - x_s / x_fq resident in SBUF as fp8e4 (one DRAM read per image);
    wq/wk/wv fp8 with DoubleRow mix matmuls (256-deep contraction per
    instruction), w1 bf16.
  - Per 128-channel block o: q/v mix matmuls -> staging in "pair layout"
    (channels 0-63 write w at free slots 0..55, channels 64-127 at
    64..119); k mix reuses the q staging tile after its transpose.
    Staging tiles are PERSISTENT and zero-initialized once: slots
    56:64 / 120:128 stay zero forever, so full-width [128, *] softmax
    ops are safe (garbage rows are never amplified into used lanes).
  - One xbar DMA-transpose per matrix -> QT/KT [wslot, h, c] and
    VH [hslot, w, c]; channel pair (c, 64+c) then runs attention as
    concurrent PE quadrant matmuls: A at partitions 0-55 / psum 0-55,
    B at partitions 64-119 / psum 64-119 (tile_position (0,0)/(64,64)).
  - softmax per group: ONE exp on ACT over all 128 partitions;
    denominators via ONE block-diagonal all-ones matmul (den_lo ->
    psum 0:64, den_hi -> 64:128); 1/sum via ln+exp on ACT; ONE
    pre-normalize multiply on DVE.
  - fuse -> FT2 [wslot, h, c] -> back-xbar -> fn2 [c, h, wslot] ->
    DRAM bounce in padded layout (contiguous 14KB/partition descriptors).
  - Phase C: s = x_s + x_mt via bf16 gpsimd accum DMA; fuse added with
    dual-AP adds split across DVE (lo) and GpSimd (hi); conv + bias on
    ACT; y rows written as flattened [128, 464] single-descriptor DMAs.
"""

import sys

import numpy as np

sys.path.insert(0, "/opt/trn_rl_repo")

N_CORES = 8


class Cfg:
    def __init__(self, imgs=2, cb=6, h=56, w=56, rt=8):
        self.imgs = imgs
        self.cb = cb
        self.C = cb * 128
        self.H = h
        self.W = w
        self.S = h * w
        self.RT = rt
        assert h % rt == 0
        self.NT = h // rt
        self.NS = rt * w
        assert self.NS * 4 <= 2048  # one PSUM bank
        self.GC = 8  # channel pairs per attention group
        assert 64 % self.GC == 0
        self.NGRP = 64 // self.GC
        self.scale = float((h * w * self.C) ** -0.5)
        self.HP = h + 2
        self.WP = w + 2
        assert h <= 56 and w <= 56  # pair layout needs w,h <= 64; 64+h <= 128


def build_program(cfg):
    from contextlib import ExitStack

    import concourse.bass as bass
    import concourse.mybir as mybir
    import concourse.tile as tile

    f32 = mybir.dt.float32
    bf16 = mybir.dt.bfloat16
    fp8 = mybir.dt.float8e4
    AF = mybir.ActivationFunctionType
    ALU = mybir.AluOpType
    DR = mybir.MatmulPerfMode.DoubleRow

    nc = bass.Bass()

    IM, CB, H, W, S = cfg.imgs, cfg.cb, cfg.H, cfg.W, cfg.S
    RT, NT, NS, C = cfg.RT, cfg.NT, cfg.NS, cfg.C
    HP, WP = cfg.HP, cfg.WP
    GC, NGRP = cfg.GC, cfg.NGRP

    x_s = nc.declare_dram_parameter("x_s", [IM, C, S], f32, isOutput=False)
    x_fq = nc.declare_dram_parameter("x_fq", [IM, C, S], f32, isOutput=False)
    x_mt = nc.declare_dram_parameter("x_mt", [IM, C, S], f32, isOutput=False)
    wqT = nc.declare_dram_parameter("wqT", [C, C], f32, isOutput=False)
    wkT = nc.declare_dram_parameter("wkT", [C, C], f32, isOutput=False)
    wvT = nc.declare_dram_parameter("wvT", [C, C], f32, isOutput=False)
    w1T = nc.declare_dram_parameter("w1T", [C, C], f32, isOutput=False)
    b1 = nc.declare_dram_parameter("b1", [C], f32, isOutput=False)
    y = nc.declare_dram_parameter("y", [IM, C, HP, WP], f32, isOutput=True)

    # fuse bounce buffer, padded layout: [img, ob, c, h, wslot(128)]
    fnat_d = nc.dram_tensor("fnat_d", [IM, CB, 128, H, 128], bf16)

    with tile.TileContext(nc) as tc, ExitStack() as ex:
        wpool = ex.enter_context(tc.tile_pool(name="wpool", bufs=1))
        xrpool = ex.enter_context(tc.tile_pool(name="xrpool", bufs=1))
        tp = ex.enter_context(tc.tile_pool(name="tp", bufs=2))
        ftp = ex.enter_context(tc.tile_pool(name="ftp", bufs=1))
        ep = ex.enter_context(tc.tile_pool(name="ep", bufs=2))
        yrp = ex.enter_context(tc.tile_pool(name="yrp", bufs=3))
        sbp = ex.enter_context(tc.tile_pool(name="sbp", bufs=2))
        mps = ex.enter_context(tc.tile_pool(name="mps", bufs=1, space="PSUM"))
        aps = ex.enter_context(tc.tile_pool(name="aps", bufs=3, space="PSUM"))

        # ---- resident weights / constants ----
        wq = wpool.tile([128, CB, C], fp8)
        wk = wpool.tile([128, CB, C], fp8)
        wv = wpool.tile([128, CB, C], fp8)
        w1 = wpool.tile([128, CB, C], bf16)
        for dst, src in ((wq, wqT), (wk, wkT), (wv, wvT), (w1, w1T)):
            nc.gpsimd.dma_start(
                out=dst, in_=src.rearrange("(kb p) o -> p kb o", p=128))
        b1t = wpool.tile([128, CB], f32)
        nc.gpsimd.dma_start(out=b1t, in_=b1.rearrange("(kb p) -> p kb", p=128))
        bord = wpool.tile([128, CB, WP], bf16)
        nc.vector.tensor_copy(
            out=bord,
            in_=bass.AP(tensor=b1t.tensor, offset=b1t.offset,
                        ap=[list(b1t.ap[0]), list(b1t.ap[1]), [0, WP]]))
        # block-diagonal all-ones stationary: den_lo -> psum 0:64,
        # den_hi -> psum 64:128, in ONE matmul
        onesd = wpool.tile([128, 128], bf16, name="onesd")
        nc.vector.memset(onesd, 0.0)
        nc.vector.memset(onesd[0:H, 0:64], 1.0)
        nc.vector.memset(onesd[64:64 + H, 64:128], 1.0)
        # persistent pair-layout staging (k/q share one tile; each mix's
        # transpose fires as soon as that mix's copies land, so the ring
        # starts early).  Zero-init once: slots 56:64 / 120:128 are
        # never written again.
        qsg = wpool.tile([128, H, 128], bf16, name="qsg")
        vstg = wpool.tile([128, W, 128], bf16, name="vsg")
        nc.vector.memset(qsg, 0.0)
        nc.vector.memset(vstg, 0.0)
        # zero the never-written partition rows of the rotating scores
        # psum banks ONCE: matmul start= only resets addressed rows, so
        # these stay 0 and exp() of them feeds the onesd contraction a
        # harmless 1.0 instead of potential uninitialized-psum NaN.
        for _i in range(3):
            tz = aps.tile([128, GC * H], f32, tag="att")
            nc.vector.memset(tz[32:64], 0.0)
            nc.vector.memset(tz[96:128], 0.0)

        for img in range(IM):
            # ---- resident x (fp8) ----
            with nc.named_scope("xload"):
                xrs = xrpool.tile([128, CB, S], fp8, tag="xs",
                                  name=f"xrs_{img}")
                xrf = xrpool.tile([128, CB, S], fp8, tag="xf",
                                  name=f"xrf_{img}")
                xs_r = x_s[img].rearrange("(kb p) s -> p kb s", p=128)
                xf_r = x_fq[img].rearrange("(kb p) s -> p kb s", p=128)
                # chunked so the o=0 mix matmuls can start immediately
                for n in range(NT):
                    nsl = slice(n * NS, (n + 1) * NS)
                    nc.gpsimd.dma_start(out=xrs[:, :, nsl], in_=xs_r[:, :, nsl])
                    nc.gpsimd.dma_start(out=xrf[:, :, nsl], in_=xf_r[:, :, nsl])

            for o in range(CB):
                osl = slice(o * 128, (o + 1) * 128)
                # ---- phase A: channel mix into pair-layout staging ----
                with nc.named_scope("mixqv"):
                    for n in range(NT):
                        nsl = slice(n * NS, (n + 1) * NS)
                        rsl = slice(n * RT, (n + 1) * RT)
                        for ti, (wm, xr, tag) in enumerate(
                                ((wq, xrs, "qp"), (wv, xrf, "vp"))):
                            ps = mps.tile([128, NS], f32, tag=tag,
                                          bufs=2 if ti == 0 else 1)
                            for kb in range(0, CB, 2):
                                nc.tensor.matmul(
                                    ps, lhsT=wm[:, kb:kb + 2, osl],
                                    rhs=xr[:, kb:kb + 2, nsl],
                                    perf_mode=DR,
                                    start=(kb == 0), stop=(kb == CB - 2))
                            if ti == 0:
                                nc.any.tensor_copy(
                                    out=qsg[0:64, rsl, 0:W],
                                    in_=ps[0:64].rearrange(
                                        "p (r w) -> p r w", r=RT))
                                nc.any.tensor_copy(
                                    out=qsg[64:128, rsl, 64:64 + W],
                                    in_=ps[64:128].rearrange(
                                        "p (r w) -> p r w", r=RT))
                            else:
                                nc.any.tensor_copy(
                                    out=vstg[0:64, 0:W, rsl],
                                    in_=ps[0:64].rearrange(
                                        "p (r w) -> p w r", r=RT))
                                nc.any.tensor_copy(
                                    out=vstg[64:128, 0:W,
                                             64 + n * RT:64 + (n + 1) * RT],
                                    in_=ps[64:128].rearrange(
                                        "p (r w) -> p w r", r=RT))
                with nc.named_scope("xpose"):
                    QT = tp.tile([128, H, 128], bf16, tag="qt",
                                 name=f"qt_{img}_{o}")
                    VH = tp.tile([128, W, 128], bf16, tag="vh", bufs=1,
                                 name=f"vh_{img}_{o}")
                    nc.sync.dma_start(
                        out=QT, in_=qsg.rearrange("p a b -> p (a b)"),
                        transpose=True)
                    nc.sync.dma_start(
                        out=VH, in_=vstg.rearrange("p a b -> p (a b)"),
                        transpose=True)
                # k mix reuses the q staging tile (waits for QT transpose)
                with nc.named_scope("mixk"):
                    for n in range(NT):
                        nsl = slice(n * NS, (n + 1) * NS)
                        rsl = slice(n * RT, (n + 1) * RT)
                        ps = mps.tile([128, NS], f32, tag="vp")
                        for kb in range(0, CB, 2):
                            nc.tensor.matmul(
                                ps, lhsT=wk[:, kb:kb + 2, osl],
                                rhs=xrf[:, kb:kb + 2, nsl],
                                perf_mode=DR,
                                start=(kb == 0), stop=(kb == CB - 2))
                        nc.any.tensor_copy(
                            out=qsg[0:64, rsl, 0:W],
                            in_=ps[0:64].rearrange("p (r w) -> p r w", r=RT))
                        nc.any.tensor_copy(
                            out=qsg[64:128, rsl, 64:64 + W],
                            in_=ps[64:128].rearrange("p (r w) -> p r w", r=RT))
                with nc.named_scope("xposek"):
                    KT = tp.tile([128, H, 128], bf16, tag="kt",
                                 name=f"kt_{img}_{o}")
                    # NOTE: all xbar transposes must stay on ONE HWDGE ring —
                    # concurrent transposes from both rings corrupt data
                    # (shared xbar unit; verified on HW).
                    nc.sync.dma_start(
                        out=KT, in_=qsg.rearrange("p a b -> p (a b)"),
                        transpose=True)

                # ---- phase B: paired per-channel attention ----
                FT2 = ftp.tile([128, H, 128], bf16, tag="ft",
                               name=f"ft_{img}_{o}")
                with nc.named_scope("attn"):
                    for g in range(NGRP):
                        sp = aps.tile([128, GC * H], f32, tag="att")
                        for ci in range(GC):
                            j = g * GC + ci
                            csl = slice(ci * H, (ci + 1) * H)
                            nc.tensor.matmul(
                                sp[0:W, csl], lhsT=KT[0:W, :, j],
                                rhs=QT[0:W, :, j], start=True, stop=True)
                            nc.tensor.matmul(
                                sp[64:64 + W, csl], lhsT=KT[64:64 + W, :, 64 + j],
                                rhs=QT[64:64 + W, :, 64 + j],
                                start=True, stop=True)
                        # one exp over all 128 partitions: rows 56:64 /
                        # 120:128 hold stale psum (finite); exp of them is
                        # never read by bp (onesd rows are 0 there) nor by
                        # the fuse matmuls (contraction 0:56 / 64:120).
                        et = ep.tile([128, GC * H], bf16, tag="et", bufs=1)
                        nc.scalar.activation(
                            out=et, in_=sp, func=AF.Exp, scale=cfg.scale)
                        bp = aps.tile([128, GC * H], f32, tag="bp", bufs=2)
                        nc.tensor.matmul(bp, lhsT=onesd, rhs=et,
                                         start=True, stop=True)
                        # 1/sum via exp(-ln(sum)) on ACT; second step in
                        # place (tolerance here is ~2e-2)
                        rt = ep.tile([128, GC * H], f32, tag="rt", bufs=1)
                        nc.scalar.activation(out=rt, in_=bp, func=AF.Ln)
                        nc.scalar.activation(out=rt, in_=rt, func=AF.Exp,
                                             scale=-1.0)
                        en = ep.tile([128, GC * H], bf16, tag="en", bufs=1)
                        nc.vector.tensor_tensor(
                            out=en, in0=et, in1=rt, op=ALU.mult)
                        fp = aps.tile([128, GC * H], f32, tag="att")
                        for ci in range(GC):
                            j = g * GC + ci
                            csl = slice(ci * H, (ci + 1) * H)
                            nc.tensor.matmul(
                                fp[0:W, csl], lhsT=VH[0:H, :, j],
                                rhs=en[0:H, csl], start=True, stop=True)
                            nc.tensor.matmul(
                                fp[64:64 + W, csl], lhsT=VH[64:64 + H, :, 64 + j],
                                rhs=en[64:64 + H, csl], start=True, stop=True)
                        # contiguous-out cast (c is FT2's innermost dim)
                        nc.any.tensor_copy(
                            out=FT2[0:W, :, g * GC:(g + 1) * GC],
                            in_=fp[0:W].rearrange("p (c h) -> p h c", c=GC))
                        nc.any.tensor_copy(
                            out=FT2[64:64 + W, :,
                                    64 + g * GC:64 + (g + 1) * GC],
                            in_=fp[64:64 + W].rearrange(
                                "p (c h) -> p h c", c=GC))
                with nc.named_scope("xback"):
                    fn2 = ftp.tile([128, H, 128], bf16, tag="fn",
                                   name=f"fn_{img}_{o}")
                    nc.sync.dma_start(
                        out=fn2, in_=FT2.rearrange("p a b -> p (a b)"),
                        transpose=True)
                    nc.scalar.dma_start(out=fnat_d[img, o], in_=fn2)

            # ---- phase C: s-add + conv + y assembly ----
            with nc.named_scope("conv"):
                xs_i = x_s[img].rearrange("(kb p) s -> p kb s", p=128)
                xmt_i = x_mt[img].rearrange("(kb p) s -> p kb s", p=128)
                fn_i = fnat_d[img].rearrange("kb p h w -> p kb h w")
                for n in range(NT):
                    nsl = slice(n * NS, (n + 1) * NS)
                    rsl = slice(n * RT, (n + 1) * RT)
                    s0 = tp.tile([128, CB, NS], f32, tag="qt",
                                 name=f"s0_{img}_{n}")
                    nc.sync.dma_start(out=s0, in_=xs_i[:, :, nsl])
                    nc.gpsimd.dma_start(out=s0, in_=xmt_i[:, :, nsl],
                                        accum_op=ALU.add)
                    fr2 = tp.tile([128, CB, RT, 128], bf16, tag="kt",
                                  name=f"fr_{img}_{n}")
                    nc.scalar.dma_start(out=fr2, in_=fn_i[:, :, rsl, :])
                    s0b = sbp.tile([128, CB, NS], bf16, tag="s0b", bufs=2)
                    nc.vector.tensor_tensor(
                        out=s0b[0:64].rearrange("p kb (r w) -> p kb r w", r=RT),
                        in0=s0[0:64].rearrange("p kb (r w) -> p kb r w", r=RT),
                        in1=fr2[0:64, :, :, 0:W], op=ALU.add)
                    nc.vector.tensor_tensor(
                        out=s0b[64:128].rearrange(
                            "p kb (r w) -> p kb r w", r=RT),
                        in0=s0[64:128].rearrange(
                            "p kb (r w) -> p kb r w", r=RT),
                        in1=fr2[64:128, :, :, 64:64 + W], op=ALU.add)
                    for o in range(CB):
                        pc = mps.tile([128, NS], f32, tag="qp", bufs=2)
                        for kb in range(CB):
                            nc.tensor.matmul(
                                pc, lhsT=w1[:, kb, o * 128:(o + 1) * 128],
                                rhs=s0b[:, kb, :],
                                start=(kb == 0), stop=(kb == CB - 1))
                        yr = yrp.tile([128, RT, WP], f32, tag="yr", bufs=2)
                        # border columns 0 and WP-1 <- b1
                        bcol = bord[:, o:o + 1, 0:RT].rearrange(
                            "p a b -> p b a")
                        nc.vector.tensor_copy(out=yr[:, :, 0:1], in_=bcol)
                        nc.vector.tensor_copy(
                            out=yr[:, :, WP - 1:WP], in_=bcol)
                        nc.scalar.activation(
                            out=yr[:, :, 1:1 + W],
                            in_=pc.rearrange("p (r w) -> p r w", r=RT),
                            func=AF.Identity, bias=b1t[:, o:o + 1])
                        yeng = (nc.scalar, nc.sync, nc.gpsimd)[o % 3]
                        yeng.dma_start(
                            out=y[img, o * 128:(o + 1) * 128,
                                  1 + n * RT:1 + (n + 1) * RT, :].rearrange(
                                      "c r w -> c (r w)"),
                            in_=yr.rearrange("p r w -> p (r w)"))
                for o in range(CB):
                    # bf16 bord -> f32 y needs the casting (gpsimd) path
                    yo = y[img, o * 128:(o + 1) * 128]
                    nc.gpsimd.dma_start(out=yo[:, 0, :], in_=bord[:, o, :])
                    nc.gpsimd.dma_start(out=yo[:, HP - 1, :], in_=bord[:, o, :])

    import bass_rust as _bass_rust
    _bass_rust.move_matmul_waits_to_ldweights(nc.m)
    _bass_rust.generate_event_semaphores(nc)
    return nc


_PROG_CACHE = {}


def get_program():
    if "full" not in _PROG_CACHE:
        _PROG_CACHE["full"] = build_program(Cfg())
    return _PROG_CACHE["full"]


def _prep_in_maps(x_s, x_fq, x_mt, Wq, Wk, Wv, W1, b1):
    x_s = np.asarray(x_s, dtype=np.float32)
    x_fq = np.asarray(x_fq, dtype=np.float32)
    x_mt = np.asarray(x_mt, dtype=np.float32)
    wqT = np.ascontiguousarray(np.asarray(Wq, np.float32).T)
    wkT = np.ascontiguousarray(np.asarray(Wk, np.float32).T)
    wvT = np.ascontiguousarray(np.asarray(Wv, np.float32).T)
    w1T = np.ascontiguousarray(np.asarray(W1, np.float32).T)
    b1 = np.asarray(b1, dtype=np.float32)

    B, C, H, W = x_s.shape
    per = B // N_CORES
    in_maps = []
    for i in range(N_CORES):
        sl = slice(i * per, (i + 1) * per)
        in_maps.append({
            "x_s": np.ascontiguousarray(x_s[sl].reshape(per, C, H * W)),
            "x_fq": np.ascontiguousarray(x_fq[sl].reshape(per, C, H * W)),
            "x_mt": np.ascontiguousarray(x_mt[sl].reshape(per, C, H * W)),
            "wqT": wqT, "wkT": wkT, "wvT": wvT, "w1T": w1T, "b1": b1,
        })
    return in_maps, per, C, H, W


def kernel(x_s, x_fq, x_mt, Wq, Wk, Wv, W1, b1, trace=False):
    from concourse.bass_utils import run_bass_kernel_spmd

    in_maps, per, C, H, W = _prep_in_maps(
        x_s, x_fq, x_mt, Wq, Wk, Wv, W1, b1)
    nc = get_program()
    r = run_bass_kernel_spmd(nc, in_maps, list(range(N_CORES)), trace=trace)
    out = np.concatenate(
        [r.results[i]["y"].reshape(per, C, H + 2, W + 2)
         for i in range(N_CORES)], axis=0).astype(np.float32)
    if trace:
        return out, r
    return out
